# revision 4
# baseline (speedup 1.0000x reference)
"""Trainium2 Bass kernel for nn_MinibatchDiscriminator (N=512, INSIZE=512, K=64, D=16).

out = concat(x, o_b), o_b[i,k] = sum_{j!=i} exp(-sum_d |feat[i,k,d]-feat[j,k,d]|),
feat = x @ W.T + b.

8 NeuronCores, SPMD, no collectives. Host feeds each core a ROLLED view of x
(own 64 rows first) as HOST-TRANSPOSED bf16 xT, plus host-transposed bf16 wT
(no device transposes). featT columns 0..63 serve as the per-row scalar
columns; the diagonal j==i cancels exactly -> exp(0) = 1 -> subtract 1.

WINDOW=320: symmetry split - each core computes columns [0, 320) of its
rolled frame (own block + 3 full neighbor blocks + a k-masked half of the
shift-4 block), emitting row sums AND per-k column sums (extra ones-matmul
per group into a persistent PSUM bank). Host adds the column partials into
the other cores' rows; every unordered pair {i,j} is computed once fleetwide.
The k-mask is folded into psn by a tiny PE matmul adding -/+256 on masked
partitions over the shift-4 columns, so one Exp per group suffices.

Phase 1, per group of 2 rows i=2g,2g+1: 16 ad tiles [128=(8k x 16d), W]:
  - min-form chunks (DVE bf16 4x-mode tensor_scalar min; Pool bf16 min):
    |a-b| = a + b - 2 min(a,b); PE block-diag-ones matmuls accumulate
    sum_d min; a half-matmul adds -B/2 (B[(s,k),j] = sum_d featT); the Exp
    uses scale=+2 and bias=-2v with v an exact bitwise replica of the PSUM
    arithmetic at the diagonal column (so exp(arg)=1 exactly at j==i).
  - abs-form chunks (ACT activation Abs, fp8 out): direct |a-b|, consumed by
    fp8 DoubleRow matmuls (2 chunks/instruction); scale=-1, bias=0.
One ACT Exp per group computes exp(scale*psum + bias), accumulating the row
sum over j (accum_out).
"""
import sys

import numpy as np

sys.path.insert(0, "/opt/trn_rl_repo")

import ml_dtypes

import concourse.bass as bass
import concourse.tile as tile
from concourse import bacc, mybir
from concourse.bass_utils import run_bass_kernel_spmd

AF = mybir.ActivationFunctionType
OP = mybir.AluOpType
MMODE = mybir.MatmulPerfMode
FP32 = mybir.dt.float32
FP32R = mybir.dt.float32r
BF16 = mybir.dt.bfloat16
FP8 = mybir.dt.float8e4

N, INSIZE, K, D = 512, 512, 64, 16
KD = K * D
NCORES = 8
NL = N // NCORES  # 64 rows per core
P = 128
CH = KD // P      # 8 chunks of (8 k x 16 d)
NT = INSIZE // P  # 4 contraction tiles
NG = NL // 2      # 32 groups of 2 rows
BIG = 128.0       # mask magnitude (exact in fp8e4m3, max finite 240)

# Engine assignment per chunk-pair (s, cp); cp covers chunks (2cp, 2cp+1).
#   DVB: DVE, min-form bf16 (4x mode), 2 single bf16 matmuls
#   PLB: Pool, min-form bf16, 2 single bf16 matmuls
#   ACF: ACT activation Abs, fp8 out, 1 DoubleRow matmul
PAIR_ENG = {
    (0, 0): "DVB", (0, 1): "DVB", (0, 2): "PLB", (0, 3): "ACF",
    (1, 0): "DVB", (1, 1): "DVB", (1, 2): "PLB", (1, 3): "DVB",
}

TRACE = False
_cache = {}

CFG = dict(
    window=320,
    adb_bufs=10, adf_bufs=4, cb_bufs=8,
    mmp_bufs=2, nmp_bufs=3,
    colsum_lag=4,
    pair_eng=None,
)


def _abs_sc(pair_eng):
    """(s,c) chunk combos handled in abs-form (ACF)."""
    out = set()
    for (s, cp), eng in pair_eng.items():
        if eng == "ACF":
            out.add((s, 2 * cp))
            out.add((s, 2 * cp + 1))
    return out


def _build(**over):
    cfg = dict(CFG)
    cfg.update(over)
    W_ = cfg["window"]
    sym = W_ < N
    pair_eng = cfg["pair_eng"] or PAIR_ENG
    abs_sc = _abs_sc(pair_eng)
    min_tsels = [s * 8 + c for s in range(2) for c in range(CH)
                 if (s, c) not in abs_sc]
    LAG = cfg["colsum_lag"]

    nc = bacc.Bacc("TRN2", target_bir_lowering=False)
    xt_h = nc.dram_tensor("xt", [INSIZE, W_], BF16, kind="ExternalInput").ap()
    wt_h = nc.dram_tensor("wt", [INSIZE, KD], BF16, kind="ExternalInput").ap()
    b_h = nc.dram_tensor("bmat", [P, CH], FP32, kind="ExternalInput").ap()
    iden_h = nc.dram_tensor("iden", [P, P], FP32, kind="ExternalInput").ap()
    onesb_h = nc.dram_tensor("onesb", [P, 16 * P], BF16, kind="ExternalInput").ap()
    onesf_h = nc.dram_tensor("onesf", [P, 16 * P], FP8, kind="ExternalInput").ap()
    half_h = nc.dram_tensor("halfm", [P, P], FP32R, kind="ExternalInput").ap()
    scol_h = nc.dram_tensor("scol", [P, 1], FP32, kind="ExternalInput").ap()
    rm2_h = nc.dram_tensor("rm2", [P, 1], FP32, kind="ExternalInput").ap()
    orow_h = nc.dram_tensor("orow", [NG, P], FP32, kind="ExternalOutput").ap()
    if sym:
        maskm_h = nc.dram_tensor("maskm", [P, P], FP8, kind="ExternalInput").ap()
        aone_h = nc.dram_tensor("aone", [P, NL], FP8, kind="ExternalInput").ap()
        ssel_h = nc.dram_tensor("ssel", [P, K], BF16, kind="ExternalInput").ap()
        ocol_h = nc.dram_tensor("ocol", [K, W_ - NL], FP32, kind="ExternalOutput").ap()

    with tile.TileContext(nc) as tc:
        with (
            tc.tile_pool(name="const", bufs=1) as cst,
            tc.tile_pool(name="inp", bufs=1) as inp,
            tc.tile_pool(name="feat", bufs=1) as fpl,
            tc.tile_pool(name="adb", bufs=cfg["adb_bufs"]) as adbp,
            tc.tile_pool(name="adf", bufs=cfg["adf_bufs"]) as adfp,
            tc.tile_pool(name="cb", bufs=cfg["cb_bufs"]) as cbp,
            tc.tile_pool(name="tp", bufs=1, space="PSUM") as tpp,
            tc.tile_pool(name="mm", bufs=cfg["mmp_bufs"], space="PSUM") as mmp,
            tc.tile_pool(name="nm", bufs=cfg["nmp_bufs"], space="PSUM") as nmp,
            tc.tile_pool(name="cs", bufs=1, space="PSUM") as csp,
        ):
            # ---------------- loads (all DMA triggers on SP) ----------------
            xt_sb = []
            for t_i in range(NT):
                t = inp.tile([P, W_], BF16, tag=f"xt{t_i}")
                nc.sync.dma_start(out=t, in_=xt_h[P * t_i : P * (t_i + 1), :])
                xt_sb.append(t)
            wt_sb = []
            for t_i in range(NT):
                t = inp.tile([P, KD], BF16, tag=f"wt{t_i}")
                nc.sync.dma_start(out=t, in_=wt_h[P * t_i : P * (t_i + 1), :])
                wt_sb.append(t)
            b_sb = cst.tile([P, CH], FP32, tag="b")
            nc.sync.dma_start(out=b_sb, in_=b_h)
            onesb_sb = cst.tile([P, 16 * P], BF16, tag="onesb")
            nc.sync.dma_start(out=onesb_sb, in_=onesb_h)
            onesf_sb = cst.tile([P, 16 * P], FP8, tag="onesf")
            nc.sync.dma_start(out=onesf_sb, in_=onesf_h)
            half_sb = cst.tile([P, P], FP32R, tag="halfm")
            nc.sync.dma_start(out=half_sb, in_=half_h)
            scol_sb = cst.tile([P, 1], FP32, tag="scol")
            nc.sync.dma_start(out=scol_sb, in_=scol_h)
            rm2_sb = cst.tile([P, 1], FP32, tag="rm2")
            nc.sync.dma_start(out=rm2_sb, in_=rm2_h)
            iden_sb = cst.tile([P, P], FP32, tag="iden")
            nc.sync.dma_start(out=iden_sb, in_=iden_h)
            if sym:
                maskm_sb = cst.tile([P, P], FP8, tag="maskm")
                nc.sync.dma_start(out=maskm_sb, in_=maskm_h)
                aone_sb = cst.tile([P, NL], FP8, tag="aone")
                nc.sync.dma_start(out=aone_sb, in_=aone_h)
                ssel_sb = cst.tile([P, K], BF16, tag="ssel")
                nc.sync.dma_start(out=ssel_sb, in_=ssel_h)

            # ---------------- featT / colT per chunk ----------------
            featT = []
            colT = cst.tile([P, CH * NL], FP32, tag="colT")
            colTb = cst.tile([P, CH * NL], BF16, tag="colTb")
            for c in range(CH):
                psf = mmp.tile([P, W_], FP32, tag="mm")
                for t_i in range(NT):
                    nc.tensor.matmul(
                        psf, wt_sb[t_i][:, P * c : P * (c + 1)], xt_sb[t_i],
                        start=(t_i == 0), stop=(t_i == NT - 1),
                    )
                fc = fpl.tile([P, W_], BF16, tag=f"feat{c}")
                # Pool/GPSIMD cannot read PSUM; split bias-adds DVE/ACT
                if c % 2:
                    nc.vector.tensor_scalar(
                        fc, psf, b_sb[:, c : c + 1], None, op0=OP.add
                    )
                else:
                    nc.scalar.activation(
                        fc, psf, AF.Identity, bias=b_sb[:, c : c + 1], scale=1.0
                    )
                featT.append(fc)
                (nc.gpsimd if c % 2 else nc.vector).tensor_copy(
                    colT[:, NL * c : NL * (c + 1)], fc[:, 0:NL]
                )
                (nc.vector if c % 2 else nc.gpsimd).tensor_copy(
                    colTb[:, NL * c : NL * (c + 1)], fc[:, 0:NL]
                )

            # ---------------- B / v / negA tables (min-form corrections) ----
            # B[(s,k), j] = sum_d featT_{c(k)}[(k,d), j] on min partitions.
            psB = nmp.tile([P, W_], FP32, tag="nm")
            for mi, tsel in enumerate(min_tsels):
                nc.tensor.matmul(
                    psB, onesb_sb[:, P * tsel : P * (tsel + 1)], featT[tsel % 8],
                    start=(mi == 0), stop=(mi == len(min_tsels) - 1),
                )
            B_r = cst.tile([P, W_], FP32R, tag="B")
            nc.vector.tensor_copy(B_r, psB)
            # v[p, i]: exact replica of phase-1 PSUM arithmetic at the
            # diagonal column (ones matmuls on the same bf16 bits + the same
            # half matmul on the same fp32r B values).
            psv = tpp.tile([P, NL], FP32, tag="tp")
            for mi, tsel in enumerate(min_tsels):
                c = tsel % 8
                nc.tensor.matmul(
                    psv, onesb_sb[:, P * tsel : P * (tsel + 1)],
                    colTb[:, NL * c : NL * (c + 1)],
                    start=(mi == 0), stop=False,
                )
            nc.tensor.matmul(psv, half_sb, B_r[:, 0:NL], start=False, stop=True)
            v_sb = cst.tile([P, NL], FP32, tag="v")
            nc.vector.tensor_copy(v_sb, psv)
            # negA[p, g] = -2 * v[p, 2g + (p >= 64)] on min rows, 0 on abs rows
            negA = cst.tile([P, NG], FP32, tag="negA")
            vv = v_sb.rearrange("p (g s) -> p s g", s=2)
            nc.vector.tensor_copy(negA[0:NL, :], vv[0:NL, 0, :])
            nc.vector.tensor_copy(negA[NL:P, :], vv[NL:P, 1, :])
            # rm2 = -2 on min rows, 0 on abs rows (bias must be 0 there)
            nc.vector.tensor_scalar(
                negA, negA, rm2_sb[:, 0:1], None, op0=OP.mult
            )

            # ---------------- phase 1 ----------------
            o1 = cst.tile([P, NG], FP32, tag="o1")
            if sym:
                pscol = csp.tile([K, W_ - NL], FP32, tag="cs")
            cbs = [None] * NG

            def emit_colsum(cg):
                nc.tensor.matmul(
                    pscol, ssel_sb, cbs[cg][:, NL:W_],
                    start=(cg == 0), stop=(cg == NG - 1),
                    skip_group_check=True,
                )

            for g in range(NG):
                psn = nmp.tile([P, W_], FP32, tag="nm")
                mms = []  # deferred matmuls: (sort_key, lhsT, rhs, perf_mode)
                for s in range(2):
                    i = 2 * g + s
                    for cp in range(4):
                        eng = pair_eng[(s, cp)]
                        c0 = 2 * cp
                        if eng == "ACF":
                            ad = adfp.tile([P, 2, W_], FP8, tag="adf")
                        else:
                            ad = adbp.tile([P, 2, W_], BF16, tag="adb")
                        for t_i in range(2):
                            c = c0 + t_i
                            col = colT[:, NL * c + i : NL * c + i + 1]
                            if eng == "ACF":
                                nc.scalar.activation(
                                    ad[:, t_i, :], featT[c], AF.Abs,
                                    bias=col, scale=-1.0,
                                )
                            elif eng == "PLB":
                                nc.gpsimd.tensor_scalar(
                                    ad[:, t_i, :], featT[c], col, None,
                                    op0=OP.min,
                                )
                            else:
                                nc.vector.tensor_scalar(
                                    ad[:, t_i, :], featT[c], col, None,
                                    op0=OP.min,
                                )
                        if eng == "ACF":
                            tsel = s * 8 + c0
                            lhs = onesf_sb[
                                :, P * tsel : P * (tsel + 2)
                            ].rearrange("p (t m) -> p t m", t=2)
                            mms.append((1, lhs, ad, MMODE.DoubleRow))
                        else:
                            for t_i in range(2):
                                c = c0 + t_i
                                tsel = s * 8 + c
                                mms.append(
                                    (0, onesb_sb[:, P * tsel : P * (tsel + 1)],
                                     ad[:, t_i, :], None)
                                )
                mms.sort(key=lambda m: m[0])
                for mi, (_, lhs, rhs, pm) in enumerate(mms):
                    nc.tensor.matmul(
                        psn, lhs, rhs,
                        start=(mi == 0), stop=False,
                        perf_mode=pm,
                    )
                nc.tensor.matmul(psn, half_sb, B_r, start=False, stop=not sym)
                if sym:
                    # k-mask on the shift-4 columns: -/+BIG on masked partitions
                    nc.tensor.matmul(
                        psn[:, W_ - NL : W_], maskm_sb, aone_sb,
                        start=False, stop=True,
                    )
                cb = cbp.tile([P, W_], BF16, tag="cb")
                cbs[g] = cb
                nc.scalar.activation(
                    cb, psn, AF.Exp,
                    bias=negA[:, g : g + 1], scale=scol_sb[:, 0:1],
                    accum_out=o1[:, g : g + 1],
                )
                if sym and g >= LAG:
                    emit_colsum(g - LAG)
            if sym:
                for cg in range(NG - LAG, NG):
                    emit_colsum(cg)

            # ---------------- epilogue ----------------
            # remove the j==i self term (exp(0) == 1 exactly)
            nc.vector.tensor_scalar(o1, o1, 1.0, None, op0=OP.subtract)
            pso = tpp.tile([NG, P], FP32, tag="tp2")
            nc.tensor.transpose(pso, o1, iden_sb)
            oT_sb = cst.tile([NG, P], FP32, tag="oT")
            nc.vector.tensor_copy(oT_sb, pso)
            nc.sync.dma_start(out=orow_h, in_=oT_sb)
            if sym:
                ocol_sb = cst.tile([K, W_ - NL], FP32, tag="ocol")
                nc.scalar.copy(ocol_sb, pscol)
                nc.sync.dma_start(out=ocol_h, in_=ocol_sb)

    nc.finalize()
    return nc


def _consts(pair_eng):
    abs_sc = _abs_sc(pair_eng)
    ones = np.zeros((P, 16, P), np.float32)
    for s in range(2):
        for c in range(CH):
            tsel = s * 8 + c
            for gl in range(8):
                ones[16 * gl : 16 * (gl + 1), tsel, 64 * s + 8 * c + gl] = 1.0
    ones = np.ascontiguousarray(ones.reshape(P, 16 * P))
    iden = np.eye(P, dtype=np.float32)
    minrow = np.ones(P, bool)
    for s, c in abs_sc:
        minrow[64 * s + 8 * c : 64 * s + 8 * c + 8] = False
    half = np.zeros((P, P), np.float32)
    half[np.arange(P), np.arange(P)] = np.where(minrow, -0.5, 0.0)
    scol = np.where(minrow, 2.0, -1.0).astype(np.float32).reshape(P, 1)
    # ssel[(s,k), k] = 1 (sum over s)
    ssel = np.zeros((P, K), np.float32)
    ssel[np.arange(P), np.arange(P) % K] = 1.0
    return ones, iden, half, scol, ssel, minrow


def _shared(W, b, window, pair_eng):
    ones, iden, half, scol, ssel, minrow = _consts(pair_eng)
    sh = {
        "wt": np.ascontiguousarray(W.T).astype(ml_dtypes.bfloat16),
        "bmat": np.ascontiguousarray(b.reshape(CH, P).T),
        "iden": iden,
        "onesb": ones.astype(ml_dtypes.bfloat16),
        "onesf": ones.astype(ml_dtypes.float8_e4m3),
        "halfm": half,
        "scol": scol,
        "rm2": np.where(minrow, -2.0, 0.0).astype(np.float32).reshape(P, 1),
        "_minrow": minrow,
    }
    if window < N:
        sh["ssel"] = ssel.astype(ml_dtypes.bfloat16)
        sh["aone"] = np.ones((P, NL), np.float32).astype(ml_dtypes.float8_e4m3)
    return sh


def _in_map(x, shared, core, window):
    xr = np.roll(x, -NL * core, axis=0)
    m = {k: v for k, v in shared.items() if not k.startswith("_")}
    m["xt"] = np.ascontiguousarray(xr.T[:, :window]).astype(ml_dtypes.bfloat16)
    if window < N:
        minrow = shared["_minrow"]
        keep_lo = core < (NCORES // 2)
        kk = np.arange(P) % K
        keep = (kk < K // 2) if keep_lo else (kk >= K // 2)
        maskm = np.zeros((P, P), np.float32)
        diag = np.where(keep, 0.0, np.where(minrow, -BIG, BIG))
        maskm[np.arange(P), np.arange(P)] = diag
        m["maskm"] = maskm.astype(ml_dtypes.float8_e4m3)
    return m


def kernel(x, W, b):
    x = np.ascontiguousarray(np.asarray(x, np.float32))
    W = np.ascontiguousarray(np.asarray(W, np.float32))
    b = np.asarray(b, np.float32)
    window = CFG["window"]
    pair_eng = CFG["pair_eng"] or PAIR_ENG
    if "nc" not in _cache:
        _cache["nc"] = _build()
    nc = _cache["nc"]
    shared = _shared(W, b, window, pair_eng)
    in_maps = [_in_map(x, shared, c, window) for c in range(NCORES)]
    res = run_bass_kernel_spmd(
        nc, in_maps, core_ids=list(range(NCORES)), trace=TRACE
    )
    _cache["last_results"] = res
    o_b = np.zeros((N, K), np.float32)
    for c in range(NCORES):
        orow = np.asarray(res.results[c]["orow"])  # [NG, P]
        o_b[NL * c : NL * (c + 1)] += orow.reshape(NL, K)
        if window < N:
            ocol = np.asarray(res.results[c]["ocol"])  # [K, window-NL]
            rows = (NL * c + NL + np.arange(window - NL)) % N
            o_b[rows] += ocol.T
    return np.ascontiguousarray(np.concatenate([x, o_b], axis=1))


# revision 8
# speedup vs baseline: 4.7699x; 4.7699x over previous
"""Trainium2 Bass kernel for nn_MinibatchDiscriminator (N=512, INSIZE=512, K=64, D=16).

out = concat(x, o_b), o_b[i,k] = sum_{j!=i} exp(-sum_d |feat[i,k,d]-feat[j,k,d]|),
feat = x @ W.T + b.

8 NeuronCores, SPMD, no collectives. Host feeds each core a ROLLED view of x
(own 64 rows first) as HOST-TRANSPOSED bf16 xT, plus host-transposed bf16 wT
(no device transposes). featT columns 0..63 serve as the per-row scalar
columns; the diagonal j==i cancels exactly -> exp(0) = 1 -> subtract 1.

WINDOW=320: symmetry split - each core computes columns [0, 320) of its
rolled frame (own block + 3 full neighbor blocks + a k-masked half of the
shift-4 block), emitting row sums AND per-k column sums (extra ones-matmul
per group into a persistent PSUM bank). Host adds the column partials into
the other cores' rows; every unordered pair {i,j} is computed once fleetwide.
The k-mask is folded into psn by a tiny PE matmul adding -/+256 on masked
partitions over the shift-4 columns, so one Exp per group suffices.

Phase 1, per group of 2 rows i=2g,2g+1: 16 ad tiles [128=(8k x 16d), W]:
  - min-form chunks (DVE bf16 4x-mode tensor_scalar min; Pool bf16 min):
    |a-b| = a + b - 2 min(a,b); PE block-diag-ones matmuls accumulate
    sum_d min; a half-matmul adds -B/2 (B[(s,k),j] = sum_d featT); the Exp
    uses scale=+2 and bias=-2v with v an exact bitwise replica of the PSUM
    arithmetic at the diagonal column (so exp(arg)=1 exactly at j==i).
  - abs-form chunks (ACT activation Abs, fp8 out): direct |a-b|, consumed by
    fp8 DoubleRow matmuls (2 chunks/instruction); scale=-1, bias=0.
One ACT Exp per group computes exp(scale*psum + bias), accumulating the row
sum over j (accum_out).
"""
import sys

import numpy as np

sys.path.insert(0, "/opt/trn_rl_repo")

import ml_dtypes

import concourse.bass as bass
import concourse.tile as tile
from concourse import bacc, mybir
from concourse.bass_utils import run_bass_kernel_spmd

AF = mybir.ActivationFunctionType
OP = mybir.AluOpType
MMODE = mybir.MatmulPerfMode
FP32 = mybir.dt.float32
FP32R = mybir.dt.float32r
BF16 = mybir.dt.bfloat16
FP8 = mybir.dt.float8e4

N, INSIZE, K, D = 512, 512, 64, 16
KD = K * D
NCORES = 8
NL = N // NCORES  # 64 rows per core
P = 128
CH = KD // P      # 8 chunks of (8 k x 16 d)
NT = INSIZE // P  # 4 contraction tiles
NG = NL // 2      # 32 groups of 2 rows
BIG = 128.0       # mask magnitude (exact in fp8e4m3, max finite 240)

# Engine assignment per chunk-pair (s, cp); cp covers chunks (2cp, 2cp+1).
#   DVB: DVE, min-form bf16 (4x mode), 2 single bf16 matmuls
#   PLB: Pool, min-form bf16, 2 single bf16 matmuls
#   ACF: ACT activation Abs, fp8 out, 1 DoubleRow matmul
#   PLR: Pool, relu-split fp8: t1=max(a-col,0), t2=min(a-col,0) per chunk;
#        |a-b| = sum t1 - sum t2 via one [ones|-ones] DoubleRow matmul/chunk
PAIR_ENG = {
    (0, 0): "DVB", (0, 1): "DVB", (0, 2): "DVB", (0, 3): "ACF",
    (1, 0): "DVB", (1, 1): "DVB", (1, 2): "DVB", (1, 3): "ACF",
}

TRACE = False
_cache = {}

CFG = dict(
    window=320,
    adb_bufs=8, adf_bufs=6, cb_bufs=8,
    mmp_bufs=2, nmp_bufs=3,
    colsum_lag=4,
    pair_eng=None,
)


def _abs_sc(pair_eng):
    """(s,c) chunk combos handled in abs-form (ACF)."""
    out = set()
    for (s, cp), eng in pair_eng.items():
        if eng == "ACF":
            out.add((s, 2 * cp))
            out.add((s, 2 * cp + 1))
    return out


def _build(**over):
    cfg = dict(CFG)
    cfg.update(over)
    W_ = cfg["window"]
    sym = W_ < N
    pair_eng = cfg["pair_eng"] or PAIR_ENG
    abs_sc = _abs_sc(pair_eng)
    min_tsels = [s * 8 + c for s in range(2) for c in range(CH)
                 if (s, c) not in abs_sc]
    LAG = cfg["colsum_lag"]

    nc = bacc.Bacc("TRN2", target_bir_lowering=False)
    xt_h = nc.dram_tensor("xt", [INSIZE, W_], BF16, kind="ExternalInput").ap()
    wt_h = nc.dram_tensor("wt", [INSIZE, KD], BF16, kind="ExternalInput").ap()
    b_h = nc.dram_tensor("bmat", [P, CH], FP32, kind="ExternalInput").ap()
    iden_h = nc.dram_tensor("iden", [P, P], FP32, kind="ExternalInput").ap()
    onesb_h = nc.dram_tensor("onesb", [P, 16 * P], FP32R, kind="ExternalInput").ap()
    onesf_h = nc.dram_tensor("onesf", [P, 16 * P], FP8, kind="ExternalInput").ap()
    half_h = nc.dram_tensor("halfm", [P, P], FP32R, kind="ExternalInput").ap()
    scol_h = nc.dram_tensor("scol", [P, 1], FP32, kind="ExternalInput").ap()
    rm2_h = nc.dram_tensor("rm2", [P, 1], FP32, kind="ExternalInput").ap()
    orow_h = nc.dram_tensor("orow", [NG, P], FP32, kind="ExternalOutput").ap()
    if sym:
        maskm_h = nc.dram_tensor("maskm", [P, P], FP8, kind="ExternalInput").ap()
        aone_h = nc.dram_tensor("aone", [P, NL], FP8, kind="ExternalInput").ap()
        ssel_h = nc.dram_tensor("ssel", [P, K], BF16, kind="ExternalInput").ap()
        ocol_h = nc.dram_tensor("ocol", [K, W_ - NL], FP32, kind="ExternalOutput").ap()

    with tile.TileContext(nc) as tc:
        with (
            tc.tile_pool(name="const", bufs=1) as cst,
            tc.tile_pool(name="inp", bufs=1) as inp,
            tc.tile_pool(name="feat", bufs=1) as fpl,
            tc.tile_pool(name="adb", bufs=cfg["adb_bufs"]) as adbp,
            tc.tile_pool(name="adf", bufs=cfg["adf_bufs"]) as adfp,
            tc.tile_pool(name="cb", bufs=cfg["cb_bufs"]) as cbp,
            tc.tile_pool(name="tp", bufs=1, space="PSUM") as tpp,
            tc.tile_pool(name="mm", bufs=cfg["mmp_bufs"], space="PSUM") as mmp,
            tc.tile_pool(name="nm", bufs=cfg["nmp_bufs"], space="PSUM") as nmp,
            tc.tile_pool(name="cs", bufs=1, space="PSUM") as csp,
        ):
            # ---------------- loads (all DMA triggers on SP) ----------------
            xt_sb = []
            for t_i in range(NT):
                t = inp.tile([P, W_], BF16, tag=f"xt{t_i}")
                nc.sync.dma_start(out=t, in_=xt_h[P * t_i : P * (t_i + 1), :])
                xt_sb.append(t)
            wt_sb = []
            for t_i in range(NT):
                t = inp.tile([P, KD], BF16, tag=f"wt{t_i}")
                nc.sync.dma_start(out=t, in_=wt_h[P * t_i : P * (t_i + 1), :])
                wt_sb.append(t)
            b_sb = cst.tile([P, CH], FP32, tag="b")
            nc.sync.dma_start(out=b_sb, in_=b_h)
            onesb_sb = cst.tile([P, 16 * P], FP32R, tag="onesb")
            nc.sync.dma_start(out=onesb_sb, in_=onesb_h)
            onesf_sb = cst.tile([P, 16 * P], FP8, tag="onesf")
            nc.sync.dma_start(out=onesf_sb, in_=onesf_h)
            half_sb = cst.tile([P, P], FP32R, tag="halfm")
            nc.sync.dma_start(out=half_sb, in_=half_h)
            scol_sb = cst.tile([P, 1], FP32, tag="scol")
            nc.sync.dma_start(out=scol_sb, in_=scol_h)
            rm2_sb = cst.tile([P, 1], FP32, tag="rm2")
            nc.sync.dma_start(out=rm2_sb, in_=rm2_h)
            iden_sb = cst.tile([P, P], FP32, tag="iden")
            nc.sync.dma_start(out=iden_sb, in_=iden_h)
            if sym:
                maskm_sb = cst.tile([P, P], FP8, tag="maskm")
                nc.sync.dma_start(out=maskm_sb, in_=maskm_h)
                aone_sb = cst.tile([P, NL], FP8, tag="aone")
                nc.sync.dma_start(out=aone_sb, in_=aone_h)
                ssel_sb = cst.tile([P, K], BF16, tag="ssel")
                nc.sync.dma_start(out=ssel_sb, in_=ssel_h)

            # ---------------- featT / colT per chunk ----------------
            featT = []
            for c in range(CH):
                psf = mmp.tile([P, W_], FP32, tag="mm")
                for t_i in range(NT):
                    nc.tensor.matmul(
                        psf, wt_sb[t_i][:, P * c : P * (c + 1)], xt_sb[t_i],
                        start=(t_i == 0), stop=(t_i == NT - 1),
                    )
                fc = fpl.tile([P, W_], FP32R, tag=f"feat{c}")
                # Pool/GPSIMD cannot read PSUM; split bias-adds DVE/ACT
                if c % 2:
                    nc.vector.tensor_scalar(
                        fc, psf, b_sb[:, c : c + 1], None, op0=OP.add
                    )
                else:
                    nc.scalar.activation(
                        fc, psf, AF.Identity, bias=b_sb[:, c : c + 1], scale=1.0
                    )
                featT.append(fc)

            # ---------------- B / v / negA tables (min-form corrections) ----
            # B[(s,k), j] = sum_d featT_{c(k)}[(k,d), j] on min partitions.
            psB = nmp.tile([P, W_], FP32, tag="nm")
            for mi, tsel in enumerate(min_tsels):
                nc.tensor.matmul(
                    psB, onesb_sb[:, P * tsel : P * (tsel + 1)], featT[tsel % 8],
                    start=(mi == 0), stop=(mi == len(min_tsels) - 1),
                )
            B_r = cst.tile([P, W_], FP32R, tag="B")
            nc.vector.tensor_copy(B_r, psB)
            # v[p, i]: exact replica of phase-1 PSUM arithmetic at the
            # diagonal column (ones matmuls on the same bf16 bits + the same
            # half matmul on the same fp32r B values).
            VW = 4 * NL  # fp32r moving dim must be >= 256
            psv = tpp.tile([P, VW], FP32, tag="tp")
            for mi, tsel in enumerate(min_tsels):
                c = tsel % 8
                nc.tensor.matmul(
                    psv, onesb_sb[:, P * tsel : P * (tsel + 1)],
                    featT[c][:, 0:VW],
                    start=(mi == 0), stop=False,
                )
            nc.tensor.matmul(psv, half_sb, B_r[:, 0:VW], start=False, stop=True)
            v_sb = cst.tile([P, NL], FP32, tag="v")
            nc.vector.tensor_copy(v_sb, psv[:, 0:NL])
            # negA[p, g] = -2 * v[p, 2g + (p >= 64)] on min rows, 0 on abs rows
            negA = cst.tile([P, NG], FP32, tag="negA")
            vv = v_sb.rearrange("p (g s) -> p s g", s=2)
            nc.vector.tensor_copy(negA[0:NL, :], vv[0:NL, 0, :])
            nc.vector.tensor_copy(negA[NL:P, :], vv[NL:P, 1, :])
            # rm2 = -2 on min rows, 0 on abs rows (bias must be 0 there)
            nc.vector.tensor_scalar(
                negA, negA, rm2_sb[:, 0:1], None, op0=OP.mult
            )

            # ---------------- phase 1 ----------------
            o1 = cst.tile([P, NG], FP32, tag="o1")
            if sym:
                pscol = csp.tile([K, W_ - NL], FP32, tag="cs")
            cbs = [None] * NG

            def emit_colsum(cg):
                nc.tensor.matmul(
                    pscol, ssel_sb, cbs[cg][:, NL:W_],
                    start=(cg == 0), stop=(cg == NG - 1),
                    skip_group_check=True,
                )

            for g in range(NG):
                psn = nmp.tile([P, W_], FP32, tag="nm")
                mms = []  # deferred matmuls: (sort_key, lhsT, rhs, perf_mode)
                for s in range(2):
                    i = 2 * g + s
                    for cp in range(4):
                        eng = pair_eng[(s, cp)]
                        c0 = 2 * cp
                        if eng == "ACF":
                            ad = adfp.tile([P, 2, W_], FP8, tag="adf")
                        else:
                            ad = adbp.tile([P, 2, W_], FP32R, tag="adb")
                        for t_i in range(2):
                            c = c0 + t_i
                            col = featT[c][:, i : i + 1].bitcast(FP32)
                            if eng == "ACF":
                                nc.scalar.activation(
                                    ad[:, t_i, :], featT[c].bitcast(FP32), AF.Abs,
                                    bias=col, scale=-1.0,
                                )
                            else:
                                nc.vector.tensor_scalar(
                                    ad[:, t_i, :], featT[c], col, None,
                                    op0=OP.min,
                                )
                        if eng == "ACF":
                            tsel = s * 8 + c0
                            lhs = onesf_sb[
                                :, P * tsel : P * (tsel + 2)
                            ].rearrange("p (t m) -> p t m", t=2)
                            mms.append((1, lhs, ad, MMODE.DoubleRow))
                        else:
                            for t_i in range(2):
                                c = c0 + t_i
                                tsel = s * 8 + c
                                mms.append(
                                    (0, onesb_sb[:, P * tsel : P * (tsel + 1)],
                                     ad[:, t_i, :], None)
                                )
                mms.sort(key=lambda m: m[0])
                for mi, (_, lhs, rhs, pm) in enumerate(mms):
                    nc.tensor.matmul(
                        psn, lhs, rhs,
                        start=(mi == 0), stop=False,
                        perf_mode=pm,
                    )
                nc.tensor.matmul(psn, half_sb, B_r, start=False, stop=not sym)
                if sym:
                    # k-mask on the shift-4 columns: -/+BIG on masked partitions
                    nc.tensor.matmul(
                        psn[:, W_ - NL : W_], maskm_sb, aone_sb,
                        start=False, stop=True,
                    )
                cb = cbp.tile([P, W_], BF16, tag="cb")
                cbs[g] = cb
                nc.scalar.activation(
                    cb, psn, AF.Exp,
                    bias=negA[:, g : g + 1], scale=scol_sb[:, 0:1],
                    accum_out=o1[:, g : g + 1],
                )
                if sym and g >= LAG:
                    emit_colsum(g - LAG)
            if sym:
                for cg in range(NG - LAG, NG):
                    emit_colsum(cg)

            # ---------------- epilogue ----------------
            # remove the j==i self term (exp(0) == 1 exactly)
            nc.vector.tensor_scalar(o1, o1, 1.0, None, op0=OP.subtract)
            pso = tpp.tile([NG, P], FP32, tag="tp2")
            nc.tensor.transpose(pso, o1, iden_sb)
            oT_sb = cst.tile([NG, P], FP32, tag="oT")
            nc.vector.tensor_copy(oT_sb, pso)
            nc.sync.dma_start(out=orow_h, in_=oT_sb)
            if sym:
                ocol_sb = cst.tile([K, W_ - NL], FP32, tag="ocol")
                nc.scalar.copy(ocol_sb, pscol)
                nc.sync.dma_start(out=ocol_h, in_=ocol_sb)

    nc.finalize()
    return nc


def _consts(pair_eng):
    abs_sc = _abs_sc(pair_eng)
    ones = np.zeros((P, 16, P), np.float32)
    for s in range(2):
        for c in range(CH):
            tsel = s * 8 + c
            for gl in range(8):
                ones[16 * gl : 16 * (gl + 1), tsel, 64 * s + 8 * c + gl] = 1.0
    ones = np.ascontiguousarray(ones.reshape(P, 16 * P))
    iden = np.eye(P, dtype=np.float32)
    minrow = np.ones(P, bool)
    for s, c in abs_sc:
        minrow[64 * s + 8 * c : 64 * s + 8 * c + 8] = False
    half = np.zeros((P, P), np.float32)
    half[np.arange(P), np.arange(P)] = np.where(minrow, -0.5, 0.0)
    scol = np.where(minrow, 2.0, -1.0).astype(np.float32).reshape(P, 1)
    # ssel[(s,k), k] = 1 (sum over s)
    ssel = np.zeros((P, K), np.float32)
    ssel[np.arange(P), np.arange(P) % K] = 1.0
    return ones, iden, half, scol, ssel, minrow


def _shared(W, b, window, pair_eng):
    ones, iden, half, scol, ssel, minrow = _consts(pair_eng)
    sh = {
        "wt": np.ascontiguousarray(W.T).astype(ml_dtypes.bfloat16),
        "bmat": np.ascontiguousarray(b.reshape(CH, P).T),
        "iden": iden,
        "onesb": ones,  # fp32r dram tensor binds as float32 bits
        "onesf": ones.astype(ml_dtypes.float8_e4m3),
        "halfm": half,
        "scol": scol,
        "rm2": np.where(minrow, -2.0, 0.0).astype(np.float32).reshape(P, 1),
        "_minrow": minrow,
    }
    if window < N:
        sh["ssel"] = ssel.astype(ml_dtypes.bfloat16)
        sh["aone"] = np.ones((P, NL), np.float32).astype(ml_dtypes.float8_e4m3)
    return sh


def _in_map(x, shared, core, window):
    xr = np.roll(x, -NL * core, axis=0)
    m = {k: v for k, v in shared.items() if not k.startswith("_")}
    m["xt"] = np.ascontiguousarray(xr.T[:, :window]).astype(ml_dtypes.bfloat16)
    if window < N:
        minrow = shared["_minrow"]
        keep_lo = core < (NCORES // 2)
        kk = np.arange(P) % K
        keep = (kk < K // 2) if keep_lo else (kk >= K // 2)
        maskm = np.zeros((P, P), np.float32)
        diag = np.where(keep, 0.0, np.where(minrow, -BIG, BIG))
        maskm[np.arange(P), np.arange(P)] = diag
        m["maskm"] = maskm.astype(ml_dtypes.float8_e4m3)
    return m


def kernel(x, W, b):
    x = np.ascontiguousarray(np.asarray(x, np.float32))
    W = np.ascontiguousarray(np.asarray(W, np.float32))
    b = np.asarray(b, np.float32)
    window = CFG["window"]
    pair_eng = CFG["pair_eng"] or PAIR_ENG
    if "nc" not in _cache:
        _cache["nc"] = _build()
    nc = _cache["nc"]
    shared = _shared(W, b, window, pair_eng)
    in_maps = [_in_map(x, shared, c, window) for c in range(NCORES)]
    res = run_bass_kernel_spmd(
        nc, in_maps, core_ids=list(range(NCORES)), trace=TRACE
    )
    _cache["last_results"] = res
    o_b = np.zeros((N, K), np.float32)
    for c in range(NCORES):
        orow = np.asarray(res.results[c]["orow"])  # [NG, P]
        o_b[NL * c : NL * (c + 1)] += orow.reshape(NL, K)
        if window < N:
            ocol = np.asarray(res.results[c]["ocol"])  # [K, window-NL]
            rows = (NL * c + NL + np.arange(window - NL)) % N
            o_b[rows] += ocol.T
    return np.ascontiguousarray(np.concatenate([x, o_b], axis=1))
